# revision 36
# baseline (speedup 1.0000x reference)
"""LlamaAttention (B=2, S=2048, H=4096, NH=32) on 8 Trainium2 NeuronCores.

Tensor-parallel over heads (4 heads/core), host-side reduction of the
row-parallel Wo partials (the TP all-reduce, done during unshard).
~1.21 ms/iter on HW (baseline: 26.9 ms); PE-engine-bound in phase 1.

Scheduling notes (HWDGE ring occupancy ~1-2us SP.SEQ per dma_start makes
DMA COUNT, not bytes, a first-order cost; all transfers are laid out
>=1KB-contiguous per partition to avoid descriptor explosion):
  - wv relayout [128, 8, 4, 512]: V weights stream in 4-subtile batches
    (64 loads/iter instead of 256).
  - Phase-1 Q/K RoPE results staged in a [128, S] wide tile, one store
    per (w, dsub) instead of per chunk (16 stores vs 64).
  - Phase-1 V results staged [128, 4, 512], one store per chunk (8 vs 32).
  - Phase-3 outputs staged [128, BT], one store per oi row-strip (32 vs 256).
  - Stores issue on the ACT HWDGE ring (nc.scalar), loads on SP ring.
  - 1/sqrt(HD) folded into the exp activation's scale (mask tiles
    pre-divided host-side): q and k share one cos/sin table pair.
  - Phase-2 software pipeline across qt boundaries: block i+1's score
    matmul + exp issue ahead of block i's sum/ctx matmuls (hides the
    PE->ACT->PE roundtrip); per-qt accumulators double-buffered.
  - bf16 storage everywhere (fp32 PSUM); mask tiles deduped host-side
    (causal -> 4 unique SBUF-resident tiles).
"""
import sys

sys.path.insert(0, "/opt/trn_rl_repo")

import numpy as np

import concourse.bass as bass
import concourse.bacc as bacc
import concourse.tile as tile
import concourse.mybir as mybir

B, S, H, NH = 2, 2048, 4096, 32
HD = H // NH          # 128
NC = 8                # cores
DL = H // NC          # 512 local dims (4 heads / core)
NHL = NH // NC        # 4 local heads
BT = B * S            # 4096 tokens
P = 128
CH = 512              # phase-1 X^T chunk (matmul moving dim)
NCH = S // CH         # 4 chunks per batch
QT = 512              # phase-2 query tile (free dim)
KT = 128              # phase-2 key tile (partition dim)
NKO = H // P          # 32 contraction subtiles
NDS = DL // P         # 4 dsub tiles
NWQ = NKO // 4        # 8 wv load groups

SCALE = float(1.0 / np.sqrt(np.float32(HD)))

DT = mybir.dt.float32
BF = mybir.dt.bfloat16
F32 = mybir.dt.float32
AF = mybir.ActivationFunctionType


def _phase1_batch(nc, tc, b, pools, aps, tabs, scratch):
    """QKV projections + RoPE for batch b."""
    p1, p1w, p1s, p1r, p1c, psA, psV = pools
    xt4, wq4, wk4, wv4 = aps
    qt_d, kt_d, v_d = scratch          # [DL, S], [DL, S], [S, DL] bf16
    cosT, sinT = tabs

    # All P1 loads ride the SP ring sequentially in need-order: the first
    # dsub's weights FIRST (they gate the first matmul), then x chunk 0,
    # etc. Racing transfers on two rings would split DMA bandwidth and
    # delay the gating one. Jobs interleave q/k per dsub so head h's
    # q AND k scratch rows are both stored after 2(h+1) jobs -- phase 2's
    # k/q prefetch (SBUF-ungated via the pkq pool) can start mid-phase-1.
    jobs = [(w4, outd, dsub) for dsub in range(NDS)
            for (w4, outd) in ((wq4, qt_d), (wk4, kt_d))]
    wtiles = {}

    def loadw(j):
        w4, _, dsub = jobs[j]
        w_sb = p1w.tile([P, NKO, P], BF, tag="wqk", name=f"wqk{b}_{j}")
        nc.sync.dma_start(w_sb[:], w4[:, dsub])
        wtiles[j] = w_sb

    loadw(0)
    xch = []
    for c in range(NCH):
        xc = p1.tile([P, NKO, CH], BF, tag="xt", name=f"xt{b}_{c}")
        nc.sync.dma_start(xc[:], xt4[:, b * NCH + c])
        xch.append(xc)
    # --- Q^T and K^T with RoPE (unscaled; softmax scale folded into exp) ---
    for j, (w4, outd, dsub) in enumerate(jobs):
            if j + 1 < len(jobs):
                loadw(j + 1)
            w_sb = wtiles.pop(j)
            ro = p1r.tile([P, S], BF, tag="ro")
            for c in range(NCH):
                psum = psA.tile([P, CH], F32, tag="qk")
                for hs in range(NKO):
                    nc.tensor.matmul(
                        psum[:], w_sb[:, hs, :], xch[c][:, hs, :],
                        start=(hs == 0), stop=(hs == NKO - 1))
                tsl = bass.ds(c * CH, CH)
                rc = p1c.tile([P, CH], F32, tag="rc")
                rs = p1c.tile([P, CH], F32, tag="rs")
                nc.vector.tensor_mul(rc[:], psum[:], cosT[:, tsl])
                nc.vector.tensor_mul(
                    rs[0:64, :], psum[64:128, :], sinT[0:64, tsl])
                nc.vector.tensor_mul(
                    rs[64:128, :], psum[0:64, :], sinT[64:128, tsl])
                nc.vector.tensor_tensor(
                    ro[0:64, tsl], rc[0:64, :], rs[0:64, :],
                    mybir.AluOpType.subtract)
                nc.vector.tensor_tensor(
                    ro[64:128, tsl], rc[64:128, :], rs[64:128, :],
                    mybir.AluOpType.add)
            nc.scalar.dma_start(outd[bass.ts(dsub, P), :], ro[:])
    # --- V in [t, d] layout; forward chunk order so chunk 0's slot frees
    # as early as possible for the next batch's xc prefetch ---
    for c in range(NCH):
        psums = [psV.tile([P, DL], F32, tag="v", name=f"vps{j}")
                 for j in range(CH // P)]
        for hq in range(NWQ):
            wv_sb = p1s.tile([P, 4, DL], BF, tag="wv")
            nc.sync.dma_start(wv_sb[:], wv4[:, hq])
            for s4 in range(4):
                hs = hq * 4 + s4
                for j in range(CH // P):
                    nc.tensor.matmul(
                        psums[j][:], xch[c][:, hs, bass.ts(j, P)],
                        wv_sb[:, s4, :],
                        start=(hs == 0), stop=(hs == NKO - 1))
        vo = p1s.tile([P, CH // P, DL], BF, tag="vo")
        for j in range(CH // P):
            nc.vector.tensor_copy(vo[:, j, :], psums[j][:])
        nc.scalar.dma_start(
            v_d[bass.ds(c * CH, CH), :].rearrange("(j p) d -> p j d", p=P),
            vo[:])


def _phase2(nc, tc, specs, n_mb, pools, mtiles, ones_b, scratches, ctx_tiles):
    """Attention, both batches -> ctx_tiles[b] [P, NHL, S] bf16.

    One flat software pipeline over (batch, head, qt, kt): scores+exp
    issue LOOK blocks ahead of block i's sum/ctx matmuls. The per-block
    chain PE->(DVE mask)->ACT->PE costs ~0.7-1.4us; each block of
    lookahead buys ~640ns of PE work, so LOOK=3 keeps the PE fed even
    through masked blocks and across head/batch boundaries. Accumulator
    pairs are per-qt; bufs=2 on psSum/psC lets qt's normalization drain
    while qt+1 accumulates.
    """
    p2, p2v, p2e, psS, psSum, psC = pools

    # k/q gate the first score matmuls -> SP ring, head 0 issued first.
    # V tiles ride the idle gpsimd SWDGE ring in 4-block batches.
    kqs = {}

    def load_kq(u):
        b, h = divmod(u, NHL)
        qt_d, kt_d, _ = scratches[b]
        k_sb = p2.tile([P, S], BF, tag="k_sb", name=f"k{b}_{h}")
        nc.sync.dma_start(k_sb[:], kt_d[bass.ts(h, P), :])
        q_sb = p2.tile([P, S], BF, tag="q_sb", name=f"q{b}_{h}")
        nc.sync.dma_start(q_sb[:], qt_d[bass.ts(h, P), :])
        kqs[u] = (k_sb, q_sb)

    load_kq(0)
    vts_b = {}

    def load_v(b):
        v_d = scratches[b][2]
        vts = []
        for kg in range(S // KT // 4):
            vt4 = p2v.tile([P, 4, DL], BF, tag=f"vg{kg}", name=f"v{b}_{kg}")
            nc.gpsimd.dma_start(
                vt4[:],
                v_d[bass.ds(kg * 4 * P, 4 * P), :].rearrange(
                    "(j p) d -> p j d", p=P))
            for j in range(4):
                vts.append((vt4, j))
        vts_b[b] = vts

    load_v(0)

    flat = []
    for u in range(B * NHL):
        b = u // NHL
        spec = specs[b % n_mb]
        for qt in range(S // QT):
            blocks = spec[qt]
            for j, (kt, mi, off, mw) in enumerate(blocks):
                flat.append((u, qt, kt, mi, off, mw, j == 0,
                             j == len(blocks) - 1, len(flat)))

    head_first = {}           # u -> flat index of its first block
    for e in flat:
        head_first.setdefault(e[0], e[8])

    def score(i):
        u, qt, kt, mi, off, mw, _, _, _ = flat[i]
        if head_first.get(u + 1) is not None and i == head_first[u]:
            load_kq(u + 1)    # prefetch next head's k/q one head ahead
            if (u + 1) % NHL == 0:
                load_v((u + 1) // NHL)  # b1's V tiles, deferred to mid-P2
        k_sb, q_sb = kqs[u]
        psum_s = psS.tile([P, QT], F32, tag="s", name=f"s{i}")
        nc.tensor.matmul(
            psum_s[:, bass.ds(off, QT - off)], k_sb[:, bass.ts(kt, KT)],
            q_sb[:, bass.ds(qt * QT + off, QT - off)],
            start=True, stop=True)
        if mi is not None:
            nc.vector.tensor_tensor(
                psum_s[:, bass.ds(off, mw)], psum_s[:, bass.ds(off, mw)],
                mtiles[mi][:], mybir.AluOpType.add)
        e_sb = p2e.tile([P, QT], BF, tag="e", name=f"e{i}")
        nc.scalar.activation(e_sb[:, bass.ds(off, QT - off)],
                             psum_s[:, bass.ds(off, QT - off)],
                             AF.Exp, scale=SCALE)
        return e_sb

    LOOK = 3
    nf = len(flat)
    es = {j: score(j) for j in range(min(LOOK, nf))}
    psum_sum = psum_ctx = None
    for i, (u, qt, kt, mi, off, mw, first, last, _) in enumerate(flat):
        if i + LOOK < nf:
            es[i + LOOK] = score(i + LOOK)
        e_cur = es.pop(i)
        b, h = divmod(u, NHL)
        if first:
            psum_sum = psSum.tile([P, QT], F32, tag="sum",
                                  name=f"sum{u}_{qt}")
            psum_ctx = psC.tile([P, QT], F32, tag="ctx",
                                name=f"ctx{u}_{qt}")
        wsl = bass.ds(off, QT - off)
        nc.tensor.matmul(psum_sum[:, wsl], ones_b[:], e_cur[:, wsl],
                         start=first, stop=last, skip_group_check=True)
        vt4, vj = vts_b[b][kt]
        nc.tensor.matmul(psum_ctx[:, wsl], vt4[:, vj, bass.ts(h, P)],
                         e_cur[:, wsl],
                         start=first, stop=last, skip_group_check=True)
        if last:
            recip = p2e.tile([P, QT], F32, tag="recip",
                             name=f"recip{u}_{qt}")
            nc.vector.reciprocal(recip[:], psum_sum[:])
            nc.vector.tensor_mul(
                ctx_tiles[b][:, h, bass.ts(qt, QT)], psum_ctx[:], recip[:])
        if last and u > 0 and kqs.get(u - 1):
            del kqs[u - 1]


def _phase3(nc, tc, pools, wo4, ctx_tiles, ot):
    import os
    var = os.environ.get("K3VAR", "")
    p3w, p3o, psO = pools
    for oi in range(H // P):
        wo_sb = p3w.tile([P, NHL, P], BF, tag="wo")
        nc.sync.dma_start(wo_sb[:], wo4[:, oi])
        o_w = p3o.tile([P, BT], BF, tag="o_w")
        for b in range(B):
            ctxT = ctx_tiles[b]
            for qt in range(S // QT):
                psum_o = psO.tile([P, QT], F32, tag="o")
                for hs in range(NHL):
                    nc.tensor.matmul(
                        psum_o[:], wo_sb[:, hs, :], ctxT[:, hs, bass.ts(qt, QT)],
                        start=(hs == 0), stop=(hs == NHL - 1))
                # alternate copies between DVE and ACT: neither engine's
                # copy stream alone can keep up with the PE cadence here
                cw = 64 if var == "thin" else QT   # thin: timing diagnostic
                osl = o_w[:, bass.ds(b * S + qt * QT, cw)]
                if (b * (S // QT) + qt) % 2 == 0:
                    nc.vector.tensor_copy(osl, psum_o[:, 0:cw])
                else:
                    nc.scalar.copy(osl, psum_o[:, 0:cw])
        if var == "spstore":
            nc.sync.dma_start(ot[bass.ts(oi, P), :], o_w[:])
        elif var == "halfstore":   # timing diagnostic only
            nc.scalar.dma_start(ot[bass.ts(oi, P), 0:BT // 2],
                                o_w[:, 0:BT // 2])
        else:
            nc.scalar.dma_start(ot[bass.ts(oi, P), :], o_w[:])


def _build(specs, n_mb, mws, reps=1, phases=(1, 2, 3), staggered=False):
    n_u = len(mws)
    mwmax = max(mws, default=1)
    nc = bacc.Bacc()

    xt4 = nc.declare_dram_parameter("xt4", [P, B * NCH, NKO, CH], BF,
                                    isOutput=False)
    wq4 = nc.declare_dram_parameter("wq4", [P, NDS, NKO, P], BF, isOutput=False)
    wk4 = nc.declare_dram_parameter("wk4", [P, NDS, NKO, P], BF, isOutput=False)
    wv4 = nc.declare_dram_parameter("wv4", [P, NWQ, 4, DL], BF, isOutput=False)
    wo4 = nc.declare_dram_parameter("wo4", [P, H // P, NHL, P], BF,
                                    isOutput=False)
    masku = nc.declare_dram_parameter("masku", [max(n_u, 1), KT, mwmax],
                                      DT, isOutput=False)
    cost = nc.declare_dram_parameter("cost", [HD, S], BF, isOutput=False)
    sint = nc.declare_dram_parameter("sint", [HD, S], BF, isOutput=False)
    ot = nc.declare_dram_parameter("ot", [H, BT], BF, isOutput=True)

    import contextlib

    with tile.TileContext(nc) as tc:
        with (
            tc.tile_pool(name="glob", bufs=1) as glob,
            tc.tile_pool(name="dram", bufs=1, space="DRAM") as dram,
        ):
            scratches = []
            for b in range(B):
                qd = dram.tile([DL, S], BF, tag=f"qt_d{b}", name=f"qt_d{b}")
                kd = dram.tile([DL, S], BF, tag=f"kt_d{b}", name=f"kt_d{b}")
                vd = dram.tile([S, DL], BF, tag=f"v_d{b}", name=f"v_d{b}")
                scratches.append((qd, kd, vd))

            ones_f = glob.tile([P, P], F32, tag="ones_f")
            nc.any.memset(ones_f[:], 1.0)
            ones_b = glob.tile([P, P], BF, tag="ones_b")
            nc.vector.tensor_copy(ones_b[:], ones_f[:])

            if reps > 1:
                hints = (mybir.EngineType.PE, mybir.EngineType.Activation,
                         mybir.EngineType.DVE, mybir.EngineType.SP,
                         mybir.EngineType.Pool)
                loop_cm = tc.For_i(0, reps, 1, staggered_reset=staggered,
                                   hint_engines=hints)
            else:
                loop_cm = contextlib.nullcontext()
            # explicit staggered-reset stage boundaries aligned to the
            # phase structure: [P1(b0) | P1(b1) | P2 | P3]. Stage I of
            # iteration n+1 may overlap stage I+2..3 of iteration n, so
            # the next iteration's x/weight loads stream in during P3.
            explicit_stages = False  # auto-quarter stages win on HW
            with loop_cm:
                aps = (xt4, wq4, wk4, wv4)
                # P2's k/q tiles live in a pool allocated BELOW the p1
                # pools, so their loads are not WAR-gated on phase-1's
                # SBUF and can stream in while phase 1 finishes.
                pkq = tc.alloc_tile_pool(name="pkq", bufs=2)
                if 1 in phases:
                    with (
                        tc.tile_pool(name="p1", bufs=4) as p1,
                        tc.tile_pool(name="p1t", bufs=1) as p1t,
                        tc.tile_pool(name="p1w", bufs=2) as p1w,
                        tc.tile_pool(name="p1s", bufs=3) as p1s,
                        tc.tile_pool(name="p1r", bufs=2) as p1r,
                        tc.tile_pool(name="p1c", bufs=1) as p1c,
                        tc.tile_pool(name="psA", bufs=2, space="PSUM") as psA,
                        tc.tile_pool(name="psV", bufs=6, space="PSUM") as psV,
                    ):
                        cosT = p1t.tile([P, S], BF, tag="tab_c")
                        nc.sync.dma_start(cosT[:], cost[:, :])
                        sinT = p1t.tile([P, S], BF, tag="tab_s")
                        nc.sync.dma_start(sinT[:], sint[:, :])
                        p1pools = (p1, p1w, p1s, p1r, p1c, psA, psV)
                        for b in range(B):
                            _phase1_batch(nc, tc, b, p1pools, aps,
                                          (cosT, sinT), scratches[b])
                if 2 in phases or 3 in phases:
                    with tc.tile_pool(name="ctxp", bufs=1) as ctxp:
                        ctx_tiles = [
                            ctxp.tile([P, NHL, S], BF, tag=f"ctxT{b}",
                                      name=f"ctxT{b}")
                            for b in range(B)
                        ]
                        if 2 not in phases:
                            # timing-only subset: P3 needs the tiles written
                            for t in ctx_tiles:
                                nc.any.memset(t[:], 1.0)
                        if 2 in phases:
                            with (
                                tc.tile_pool(name="p2v", bufs=2) as p2v,
                                tc.tile_pool(name="p2m", bufs=1) as p2m,
                                tc.tile_pool(name="p2e", bufs=4) as p2e,
                                tc.tile_pool(name="psS", bufs=4, space="PSUM") as psS,
                                tc.tile_pool(name="psSum", bufs=2, space="PSUM") as psSum,
                                tc.tile_pool(name="psC", bufs=2, space="PSUM") as psC,
                            ):
                                mtiles = []
                                for i in range(n_u):
                                    mt = p2m.tile([KT, mws[i]], DT,
                                                  tag=f"m{i}", name=f"m{i}")
                                    nc.gpsimd.dma_start(
                                        mt[:], masku[i, :, :mws[i]])
                                    mtiles.append(mt)
                                _phase2(nc, tc, specs, n_mb,
                                        (pkq, p2v, p2e, psS, psSum, psC),
                                        mtiles, ones_b, scratches, ctx_tiles)
                        if 3 in phases:
                            with (
                                tc.tile_pool(name="p3w", bufs=3) as p3w,
                                tc.tile_pool(name="p3o", bufs=3) as p3o,
                                tc.tile_pool(name="psO", bufs=4, space="PSUM") as psO,
                            ):
                                _phase3(nc, tc, (p3w, p3o, psO), wo4,
                                        ctx_tiles, ot)
                pkq.release()
    nc.finalize()
    return nc


def _rope_tables():
    inv_freq = 1.0 / (10000.0 ** (np.arange(0, HD, 2, dtype=np.float32) / HD))
    t = np.arange(S, dtype=np.float32)
    freqs = np.einsum("i,j->ij", t, inv_freq)
    emb = np.concatenate([freqs, freqs], axis=-1)        # [S, HD]
    return np.cos(emb).astype(np.float32), np.sin(emb).astype(np.float32)


_CACHE = {}


def _bf16(a):
    import ml_dtypes

    return np.ascontiguousarray(a.astype(ml_dtypes.bfloat16))


def _block_spec_merged(mask, tiles, uniq):
    """Per-qt list of (kt, mask_idx|None, off, mw). `off` = leading
    fully-masked q-columns of the transposed (k, q) block: scores, exp and
    sum/ctx matmuls skip them entirely (the bulk of the causal-diagonal
    waste). [off, off+mw) is the window that still needs an additive mask;
    beyond it the block is clean. The first block of each qt row keeps
    off=0 so its start=True matmul initializes the full accumulator width.
    Unique mask windows are deduped across batches and pre-divided by the
    softmax scale (the scale is applied inside the exp activation)."""
    spec = []
    for qt in range(S // QT):
        row = []
        sub_q = mask[qt * QT:(qt + 1) * QT]
        for kt in range(S // KT):
            blk = sub_q[:, kt * KT:(kt + 1) * KT]
            if np.all(blk <= -1e8):
                continue                        # fully masked -> skip
            bt = blk.T                          # [KT, QT]
            off = 0
            if row:                             # non-first: skip dead cols
                fully = np.all(bt <= -1e8, axis=0)
                while off < QT - 1 and fully[off]:
                    off += 1
            rem = bt[:, off:]
            nzc = np.nonzero(np.any(rem != 0.0, axis=0))[0]
            if len(nzc):
                mw = int(nzc[-1]) + 1
                mt = np.ascontiguousarray(
                    (rem[:, :mw] / SCALE).astype(np.float32))
                key = (mw, mt.tobytes())
                mi = uniq.get(key)
                if mi is None:
                    mi = len(tiles)
                    uniq[key] = mi
                    tiles.append(mt)
                row.append((kt, mi, off, mw))
            else:
                row.append((kt, None, off, 0))
        assert row, "a query tile with all keys masked is not supported"
        spec.append(row)
    return spec


def _prep(hidden_states, attention_mask, Wq, Wk, Wv, Wo):
    """Host-side marshaling. Returns (in_maps, specs, n_mb, n_u)."""
    hidden_states = np.asarray(hidden_states, dtype=np.float32)
    attention_mask = np.asarray(attention_mask, dtype=np.float32)
    Wq = np.asarray(Wq, dtype=np.float32)
    Wk = np.asarray(Wk, dtype=np.float32)
    Wv = np.asarray(Wv, dtype=np.float32)
    Wo = np.asarray(Wo, dtype=np.float32)

    xt = hidden_states.reshape(BT, H).T                         # [H, BT]
    # [128, B*NCH(chunks), 32(ho), 512] contiguous per partition
    xt4 = _bf16(xt.reshape(NKO, P, B * NCH, CH).transpose(1, 2, 0, 3))

    wqT, wkT = Wq.T, Wk.T                                       # [H(in), H(out)]
    wvT, woT = Wv.T, Wo.T

    masks = attention_mask[:, 0]                                # [B, S, S]
    same = bool(np.array_equal(masks[0], masks[1])) if B == 2 else True
    n_mb = 1 if same else B
    specs = []
    tiles = []
    uniq = {}
    for i in range(n_mb):
        specs.append(_block_spec_merged(masks[i], tiles, uniq))
    n_u = len(tiles)
    mws = [t.shape[1] for t in tiles]
    mwmax = max(mws, default=1)
    masku = np.zeros((max(n_u, 1), KT, mwmax), np.float32)
    for i, t in enumerate(tiles):
        masku[i, :, :t.shape[1]] = t

    cos, sin = _rope_tables()
    cost = _bf16(cos.T)                                         # [HD, S]
    sint = _bf16(sin.T)

    in_maps = []
    for g in range(NC):
        dsl = slice(g * DL, (g + 1) * DL)
        wq4 = _bf16(wqT[:, dsl].reshape(NKO, P, NDS, P).transpose(1, 2, 0, 3))
        wk4 = _bf16(wkT[:, dsl].reshape(NKO, P, NDS, P).transpose(1, 2, 0, 3))
        wv4 = _bf16(wvT[:, dsl].reshape(NWQ, 4, P, DL).transpose(2, 0, 1, 3))
        wo4 = _bf16(woT[dsl, :].reshape(NHL, P, H // P, P).transpose(1, 2, 0, 3))
        in_maps.append({
            "xt4": xt4,
            "wq4": wq4,
            "wk4": wk4,
            "wv4": wv4,
            "wo4": wo4,
            "masku": masku,
            "cost": cost, "sint": sint,
        })
    return in_maps, specs, n_mb, mws


def kernel(hidden_states, attention_mask, Wq, Wk, Wv, Wo):
    from concourse.bass_utils import run_bass_kernel_spmd

    in_maps, specs, n_mb, mws = _prep(
        hidden_states, attention_mask, Wq, Wk, Wv, Wo)

    key = (n_mb, tuple(mws), tuple(tuple(map(tuple, s)) for s in specs))
    if key not in _CACHE:
        _CACHE[key] = _build(specs, n_mb, mws)
    nc = _CACHE[key]

    try:
        res = run_bass_kernel_spmd(nc, in_maps, list(range(NC)), trace=False)
    except Exception:
        # one retry: a wedged NeuronCore usually recovers on re-dispatch
        import time as _time
        _time.sleep(5)
        res = run_bass_kernel_spmd(nc, in_maps, list(range(NC)), trace=False)
    acc = np.zeros((H, BT), dtype=np.float32)
    for g in range(NC):
        acc += res.results[g]["ot"].astype(np.float32)
    return np.ascontiguousarray(acc.T).reshape(B, S, H)

